# revision 1
# baseline (speedup 1.0000x reference)
"""Multi-head attention Trainium2 kernel, 8-core SPMD.

Problem: x[2,4096,512], 8 heads of 64; per-head QKV proj, softmax(QK^T/8)V,
concat, output proj.

Sharding: sequence-parallel, no collectives. Core c handles batch b=c//4 and
query rows [1024*(c%4), 1024*(c%4)+1024). Each core computes K/V for the full
4096-row sequence of its batch (4x duplicated work, hidden under the ACT exp
bottleneck) and writes its own 1024x512 output slice.

Layouts (SBUF, partition dim first):
  xT   [128,4,512]   x^T chunk: partition=d%128, dsub=d//128, free=t_local
  kT   [128,4,4096]  bf16 K^T: partition p,group g -> row g*128+p = h*64+e
  qT   [128,4,1024]  bf16 Q^T, same row packing, local q cols
  v    [128,32,8,65] bf16 V augmented: [t%128, t//128, h, e(+ones col 64)]
  yT   [128,4,1024]  fp32 attention out^T, rows (h,e), local q cols
Scores are computed transposed (S^T[t,s]) so softmax needs no transposes:
exp on ACT reads score PSUM directly; the ones-column of V makes row 64 of
the PV accumulation equal the softmax denominator.
"""

import numpy as np

import concourse.bass as bass
from concourse import bacc
import concourse.mybir as mybir
import concourse.tile as tile
from concourse.bass_utils import run_bass_kernel_spmd

F32 = mybir.dt.float32
F32R = mybir.dt.float32r
BF16 = mybir.dt.bfloat16

B, S, D, H, E = 2, 4096, 512, 8, 64
NCORES = 8
QCHUNK = S // 4          # 1024 query rows per core
TCH = 512                # t-rows per phase-1 chunk
G = 3                    # score psum banks per exp instruction


def build_program():
    nc = bacc.Bacc()
    xt_d = nc.dram_tensor("xt", [D, S], F32R, kind="ExternalInput")
    wq_d = nc.dram_tensor("wq", [128, 4, 512], F32R, kind="ExternalInput")
    wk_d = nc.dram_tensor("wk", [128, 4, 512], F32R, kind="ExternalInput")
    wv_d = nc.dram_tensor("wv", [128, 4, 512], F32R, kind="ExternalInput")
    wo_d = nc.dram_tensor("wo", [128, 4, 512], F32R, kind="ExternalInput")
    bq_d = nc.dram_tensor("bq", [128, 4], F32, kind="ExternalInput")
    bk_d = nc.dram_tensor("bk", [128, 4], F32, kind="ExternalInput")
    bv_d = nc.dram_tensor("bv", [512], F32, kind="ExternalInput")
    bo_d = nc.dram_tensor("bo", [512], F32, kind="ExternalInput")
    out_d = nc.dram_tensor("out", [QCHUNK, D], F32, kind="ExternalOutput")

    # q0 is passed per-core but we cannot branch on it cheaply; instead each
    # core gets its own x already rolled so its query rows sit at rows 0:1024.
    # (host side rolls x; kernel always uses rows 0:1024 as queries)

    with tile.TileContext(nc) as tc:
        with (
            tc.tile_pool(name="const", bufs=1) as cpool,
            tc.tile_pool(name="work", bufs=3) as wpool,
            tc.tile_pool(name="pt", bufs=8) as ptpool,
            tc.tile_pool(name="ps", bufs=2, space="PSUM") as pspool,
            tc.tile_pool(name="dr", bufs=2, space="DRAM") as dpool,
        ):
            wq_s = cpool.tile([128, 4, 512], F32R, tag="wq")
            wk_s = cpool.tile([128, 4, 512], F32R, tag="wk")
            wv_s = cpool.tile([128, 4, 512], F32R, tag="wv")
            wo_s = cpool.tile([128, 4, 512], F32R, tag="wo")
            bq_s = cpool.tile([128, 4], F32, tag="bq")
            bk_s = cpool.tile([128, 4], F32, tag="bk")
            bv_r = cpool.tile([128, 512], F32, tag="bvr")
            bo_r = cpool.tile([128, 512], F32, tag="bor")
            nc.sync.dma_start(wq_s[:], wq_d[:])
            nc.sync.dma_start(wk_s[:], wk_d[:])
            nc.sync.dma_start(wv_s[:], wv_d[:])
            nc.sync.dma_start(wo_s[:], wo_d[:])
            nc.sync.dma_start(bq_s[:], bq_d[:])
            nc.sync.dma_start(bk_s[:], bk_d[:])
            nc.sync.dma_start(bv_r[:], bv_d[:].unsqueeze(0).to_broadcast((128, 512)))
            nc.sync.dma_start(bo_r[:], bo_d[:].unsqueeze(0).to_broadcast((128, 512)))

            kT = cpool.tile([128, 4, S], BF16, tag="kT")
            qT = cpool.tile([128, 4, QCHUNK], BF16, tag="qT")
            vA = cpool.tile([128, S // 128, H, E + 1], BF16, tag="vA")
            yT = cpool.tile([128, 4, QCHUNK], F32R, tag="yT")
            nc.vector.memset(vA[:, :, :, E], 1.0)

            # ---- phase 1: x -> xT chunks -> K^T, V, Q^T projections ----
            for ch in range(S // TCH):
                xT = wpool.tile([128, 4, TCH], F32R, tag="xT")
                for ds_ in range(4):
                    nc.sync.dma_start(
                        xT[:, ds_, :],
                        xt_d[ds_ * 128 : (ds_ + 1) * 128, ch * TCH : (ch + 1) * TCH],
                    )
                # K^T rows: 4 groups of 128
                for eg in range(4):
                    pk = pspool.tile([128, 512], F32, tag="small")
                    for ds_ in range(4):
                        nc.tensor.matmul(
                            pk[:, :TCH],
                            wk_s[:, ds_, eg * 128 : (eg + 1) * 128],
                            xT[:, ds_, :],
                            start=(ds_ == 0),
                            stop=(ds_ == 3),
                        )
                    nc.vector.tensor_tensor(
                        out=kT[:, eg, ch * TCH : (ch + 1) * TCH],
                        in0=pk[:, :TCH],
                        in1=bk_s[:, eg, None].to_broadcast([128, TCH]),
                        op=mybir.AluOpType.add,
                    )
                # V rows (t on partitions)
                for ts_ in range(TCH // 128):
                    pv = pspool.tile([128, 512], F32, tag="small")
                    for ds_ in range(4):
                        nc.tensor.matmul(
                            pv[:],
                            xT[:, ds_, ts_ * 128 : (ts_ + 1) * 128],
                            wv_s[:, ds_, :],
                            start=(ds_ == 0),
                            stop=(ds_ == 3),
                        )
                    nc.vector.tensor_tensor(
                        out=vA[:, ch * 4 + ts_, :, 0:E],
                        in0=pv[:].rearrange("p (h e) -> p h e", h=H),
                        in1=bv_r[:].rearrange("p (h e) -> p h e", h=H),
                        op=mybir.AluOpType.add,
                    )
                # Q^T for query chunks (local rows 0:1024 of this core's x)
                if ch < QCHUNK // TCH:
                    for eg in range(4):
                        pq = pspool.tile([128, 512], F32, tag="small")
                        for ds_ in range(4):
                            nc.tensor.matmul(
                                pq[:, :TCH],
                                wq_s[:, ds_, eg * 128 : (eg + 1) * 128],
                                xT[:, ds_, :],
                                start=(ds_ == 0),
                                stop=(ds_ == 3),
                            )
                        nc.vector.tensor_tensor(
                            out=qT[:, eg, ch * TCH : (ch + 1) * TCH],
                            in0=pq[:, :TCH],
                            in1=bq_s[:, eg, None].to_broadcast([128, TCH]),
                            op=mybir.AluOpType.add,
                        )

            # ---- phase 2: attention per head / 512-wide query chunk ----
            NT = S // 128          # 32 t-tiles
            for hp in range(H // 2):
                g = hp
                for sc in range(QCHUNK // 512):
                    pav0 = pspool.tile([128, 512], F32, tag="av")
                    pav1 = pspool.tile([128, 512], F32, tag="av")
                    for tt in range(NT):
                        psc = pspool.tile([128, 2, 512], F32, tag="sc")
                        for hh in range(2):
                            p0 = hh * 64
                            nc.tensor.matmul(
                                psc[:, hh, :],
                                kT[p0 : p0 + 64, g, tt * 128 : (tt + 1) * 128],
                                qT[p0 : p0 + 64, g, sc * 512 : (sc + 1) * 512],
                                start=True,
                                stop=True,
                            )
                        pt = ptpool.tile([128, 2, 512], BF16, tag="pt")
                        nc.scalar.activation(
                            pt[:],
                            psc[:],
                            mybir.ActivationFunctionType.Exp,
                            scale=0.125,
                        )
                        for hh, pav in ((0, pav0), (1, pav1)):
                            nc.tensor.matmul(
                                pav[0:65, :],
                                vA[:, tt, 2 * hp + hh, :],
                                pt[:, hh, :],
                                start=(tt == 0),
                                stop=(tt == NT - 1),
                            )
                    for hh, pav in ((0, pav0), (1, pav1)):
                        p0 = hh * 64
                        avs = wpool.tile([65, 512], F32, tag="avs")
                        nc.vector.tensor_copy(avs[:], pav[0:65, :])
                        rec = wpool.tile([1, 512], F32, tag="rec")
                        nc.vector.reciprocal(rec[:], avs[64:65, :])
                        rrep = wpool.tile([64, 512], F32, tag="rrep")
                        rec_d = dpool.tile([1, 512], F32, tag="recd")
                        nc.sync.dma_start(rec_d[:], rec[:])
                        nc.sync.dma_start(rrep[:], rec_d[:].to_broadcast((64, 512)))
                        nc.vector.tensor_tensor(
                            out=yT[p0 : p0 + 64, g, sc * 512 : (sc + 1) * 512],
                            in0=avs[0:64, :],
                            in1=rrep[:],
                            op=mybir.AluOpType.mult,
                        )

            # ---- phase 3: output projection ----
            for st in range(QCHUNK // 128):
                po = pspool.tile([128, 512], F32, tag="small")
                for g in range(4):
                    nc.tensor.matmul(
                        po[:],
                        yT[:, g, st * 128 : (st + 1) * 128],
                        wo_s[:, g, :],
                        start=(g == 0),
                        stop=(g == 3),
                    )
                o_s = wpool.tile([128, 512], F32, tag="osb")
                nc.vector.tensor_tensor(o_s[:], po[:], bo_r[:], mybir.AluOpType.add
                )
                nc.sync.dma_start(
                    out_d[st * 128 : (st + 1) * 128, :], o_s[:]
                )
    nc.compile()
    return nc


_NC = None


def kernel(x, Wq, bq, Wk, bk, Wv, bv, Wo, bo, **kw):
    global _NC
    x = np.asarray(x, np.float32)
    s = lambda a: np.ascontiguousarray(np.asarray(a, np.float32))
    # weight packing shared by all cores
    wq_p = s(np.transpose(Wq, (1, 0, 2)).reshape(D, 512).reshape(4, 128, 512)
             .transpose(1, 0, 2))
    wk_p = s(np.transpose(Wk, (1, 0, 2)).reshape(D, 512).reshape(4, 128, 512)
             .transpose(1, 0, 2))
    wv_p = s(np.transpose(Wv, (1, 0, 2)).reshape(D, 512).reshape(4, 128, 512)
             .transpose(1, 0, 2))
    wo_p = s(np.asarray(Wo, np.float32).reshape(4, 128, 512).transpose(1, 0, 2))
    bq_p = s(np.asarray(bq, np.float32).reshape(512).reshape(4, 128).T)
    bk_p = s(np.asarray(bk, np.float32).reshape(512).reshape(4, 128).T)
    bv_p = s(np.asarray(bv, np.float32).reshape(512))
    bo_p = s(np.asarray(bo, np.float32))

    if _NC is None:
        _NC = build_program()

    in_maps = []
    for c in range(NCORES):
        b = c // 4
        q0 = (c % 4) * QCHUNK
        xb = np.roll(x[b], -q0, axis=0)  # queries at rows 0:1024
        in_maps.append({
            "xt": np.ascontiguousarray(xb.T),
            "wq": wq_p, "wk": wk_p, "wv": wv_p, "wo": wo_p,
            "bq": bq_p, "bk": bk_p, "bv": bv_p, "bo": bo_p,
        })
    res = run_bass_kernel_spmd(_NC, in_maps, core_ids=list(range(NCORES)))
    out = np.empty((B, S, D), np.float32)
    for c in range(NCORES):
        b = c // 4
        q0 = (c % 4) * QCHUNK
        out[b, q0 : q0 + QCHUNK] = res.results[c]["out"]
    return out



# revision 13
# speedup vs baseline: 1.3463x; 1.3463x over previous
"""Multi-head attention Trainium2 kernel, 8-core SPMD (v2, fp8 DoubleRow).

Problem: x[2,4096,512], 8 heads of 64; per-head QKV proj, softmax(QK^T/8)V,
concat, output proj.

Sharding: sequence-parallel, no collectives. Core c handles batch b=c//4 and
query rows [1024*(c%4), ...+1024). Each core computes K/V for the full 4096-row
sequence of its batch; x is host-rolled so local query rows sit at 0:1024.

Speed structure:
- Q/K/V projections in f32r; K/Q converted to fp8-e4m3 with a host-side
  weight-column permutation so the PSUM->SBUF convert is partition-preserving
  and lands directly in the DoubleRow plane layout.
- Scores and PV use fp8-e4m3 DoubleRow matmuls (0.5 cyc/row): score stationary
  kT8[32,2,128] / moving qT8[32,2,512]; PV stationary vA[128,2,65] (ones column
  provides the softmax denominator) / moving pt[128,2,512].
- softmax exp is the wall; it is split across three engines by t-tile pair:
  lane A: ACT native Exp -> fp8 (scale=0.125 fused)
  lane D: DVE tensor_scalar computes e4m3 BITS of exp directly
          (Schraudolph: round(A*s+B) -> int8 -> bitcast fp8e4)
  lane P: DMA copies scores PSUM->SBUF, Pool does the same Schraudolph
- Attention is streamed in two t-groups (pairs 0-5, 6-15) so K/V projection of
  chunks 3..7 overlaps group-A attention; per-(h,sc) PV accumulates in one PSUM
  bank per group, group A drained to SBUF and merged at group-B end.
"""

import numpy as np

import concourse.bass as bass
from concourse import bacc
import concourse.mybir as mybir
import concourse.tile as tile
from concourse.bass_utils import run_bass_kernel_spmd

F32 = mybir.dt.float32
F32R = mybir.dt.float32r
BF16 = mybir.dt.bfloat16
FP8 = mybir.dt.float8e4
I8 = mybir.dt.int8
DR = mybir.MatmulPerfMode.DoubleRow
ADD = mybir.AluOpType.add
MULT = mybir.AluOpType.mult

B, S, D, H, E = 2, 4096, 512, 8, 64
NCORES = 8
QCHUNK = S // 4          # 1024 query rows per core
TCH = 512                # t-rows per projection chunk
NPAIR = S // 256         # 16 t-tile pairs
GROUP_A = list(range(6))        # chunks 0..2
GROUP_B = list(range(6, NPAIR)) # chunks 3..7

A_SCH = 1.442695041      # 8*log2(e)*0.125
B_SCH = 55.632           # calibrated: max rel err 7.2% per weight

# exp lane pattern, cycled over all 256 (h,sc,pair) tasks: A=ACT native exp,
# D=DVE Schraudolph. (Pool cannot read PSUM and DMA cannot read PSUM, so a
# Pool lane would cost DVE the same feeder op as doing the exp on DVE.)
LANES = "DADADAADADAADADAADADAADADAADADAA"  # 19 A / 13 D per 32


def build_program():
    nc = bacc.Bacc()
    xt_d = nc.dram_tensor("xt", [D, S], F32R, kind="ExternalInput")
    wq_d = nc.dram_tensor("wq", [128, 4, 512], F32R, kind="ExternalInput")
    wk_d = nc.dram_tensor("wk", [128, 4, 512], F32R, kind="ExternalInput")
    wv_d = nc.dram_tensor("wv", [128, 4, 512], F32R, kind="ExternalInput")
    wo_d = nc.dram_tensor("wo", [64, 8, 512], F32R, kind="ExternalInput")
    bq_d = nc.dram_tensor("bq", [128, 4], F32, kind="ExternalInput")
    bk_d = nc.dram_tensor("bk", [128, 4], F32, kind="ExternalInput")
    bv_d = nc.dram_tensor("bv", [512], F32, kind="ExternalInput")
    bo_d = nc.dram_tensor("bo", [512], F32, kind="ExternalInput")
    out_d = nc.dram_tensor("out", [QCHUNK, D], F32, kind="ExternalOutput")

    with tile.TileContext(nc) as tc:
        with (
            tc.tile_pool(name="const", bufs=1) as cpool,
            tc.tile_pool(name="work", bufs=3) as wpool,
            tc.tile_pool(name="xtp", bufs=2) as xpool,
            tc.tile_pool(name="psc", bufs=3, space="PSUM") as pscpool,
            tc.tile_pool(name="pav", bufs=2, space="PSUM") as pavpool,
            tc.tile_pool(name="dr", bufs=2, space="DRAM") as dpool,
        ):
            wq_s = cpool.tile([128, 4, 512], F32R, tag="wq")
            wk_s = cpool.tile([128, 4, 512], F32R, tag="wk")
            wv_s = cpool.tile([128, 4, 512], F32R, tag="wv")
            wo_s = cpool.tile([64, 8, 512], F32R, tag="wo")
            bq_s = cpool.tile([128, 4], F32, tag="bq")
            bk_s = cpool.tile([128, 4], F32, tag="bk")
            bv_r = cpool.tile([128, 512], F32, tag="bvr")
            bo_r = cpool.tile([128, 512], F32, tag="bor")
            nc.sync.dma_start(wk_s[:], wk_d[:])
            nc.sync.dma_start(bk_s[:], bk_d[:])
            nc.sync.dma_start(wv_s[:], wv_d[:])
            nc.sync.dma_start(bv_r[:], bv_d[:].unsqueeze(0).to_broadcast((128, 512)))
            nc.sync.dma_start(wq_s[:], wq_d[:])
            nc.sync.dma_start(bq_s[:], bq_d[:])
            nc.sync.dma_start(bo_r[:], bo_d[:].unsqueeze(0).to_broadcast((128, 512)))
            nc.sync.dma_start(wo_s[:], wo_d[:])

            # fp8 operand tensors
            kT8 = cpool.tile([128, 4, S], FP8, tag="kT8")
            qT8 = cpool.tile([128, 4, QCHUNK], FP8, tag="qT8")
            vA = cpool.tile([128, S // 128, H, E + 2], FP8, tag="vA")
            yacc = cpool.tile([65, 16, 512], F32, tag="yacc")
            yT2 = cpool.tile([64, H, QCHUNK], F32R, tag="yT2")
            nc.vector.memset(vA[:, :, :, E], 1.0)
            nc.vector.memset(vA[:, :, :, E + 1], 0.0)

            def make_proj_tasks(ch):
                """Per-chunk projection split into psc-ring-sized tasks so it
                can interleave with attention blocks at fine grain."""
                state = {}

                def t_x():
                    xT = xpool.tile([128, 4, TCH], F32R, tag="xT")
                    for ds in range(4):
                        nc.sync.dma_start(
                            xT[:, ds, :],
                            xt_d[ds * 128:(ds + 1) * 128,
                                 ch * TCH:(ch + 1) * TCH])
                    state["xT"] = xT

                def t_k(ega):
                    def f():
                        xT = state["xT"]
                        pk = pscpool.tile([128, 2, 512], F32, tag="psc")
                        for i in range(2):
                            eg = ega * 2 + i
                            for ds in range(4):
                                nc.tensor.matmul(
                                    pk[:, i, :],
                                    wk_s[:, ds, eg * 128:(eg + 1) * 128],
                                    xT[:, ds, :], start=(ds == 0),
                                    stop=(ds == 3))
                            nc.scalar.activation(
                                kT8[:, eg, ch * TCH:(ch + 1) * TCH],
                                pk[:, i, :],
                                mybir.ActivationFunctionType.Identity,
                                bias=bk_s[:, eg:eg + 1], scale=1.0)
                    return f

                def t_v(tsa):
                    def f():
                        xT = state["xT"]
                        pv = pscpool.tile([128, 2, 512], F32, tag="psc")
                        for i in range(2):
                            ts = tsa * 2 + i
                            for ds in range(4):
                                nc.tensor.matmul(
                                    pv[:, i, :],
                                    xT[:, ds, ts * 128:(ts + 1) * 128],
                                    wv_s[:, ds, :], start=(ds == 0),
                                    stop=(ds == 3))
                            nc.vector.tensor_tensor(
                                out=vA[:, ch * 4 + ts, :, 0:E],
                                in0=pv[:, i, :].rearrange("p (h e) -> p h e",
                                                          h=H),
                                in1=bv_r[:].rearrange("p (h e) -> p h e", h=H),
                                op=ADD)
                    return f

                def t_q(ega):
                    def f():
                        xT = state["xT"]
                        pq = pscpool.tile([128, 2, 512], F32, tag="psc")
                        for i in range(2):
                            eg = ega * 2 + i
                            for ds in range(4):
                                nc.tensor.matmul(
                                    pq[:, i, :],
                                    wq_s[:, ds, eg * 128:(eg + 1) * 128],
                                    xT[:, ds, :], start=(ds == 0),
                                    stop=(ds == 3))
                            nc.vector.tensor_scalar(
                                qT8[:, eg, ch * TCH:(ch + 1) * TCH],
                                pq[:, i, :], bq_s[:, eg:eg + 1], None, ADD)
                    return f

                def first():
                    t_x()
                    t_k(0)()

                tasks = [first, t_k(1), t_v(0), t_v(1)]
                if ch < QCHUNK // TCH:
                    tasks += [t_q(0), t_q(1)]
                return tasks

            def emit_proj_chunk(ch):
                for t in make_proj_tasks(ch):
                    t()

            lane_ctr = [0]

            def emit_block(h, sc, pairs, is_group_a):
                a0 = 32 * (h % 4)
                g0 = 2 * (h // 4)
                n = len(pairs)
                pav = pavpool.tile([128, 512], F32, tag="pav")

                def emit_pv(item):
                    j, tp, ptv = item
                    nc.tensor.matmul(
                        pav[0:66, :], vA[:, 2 * tp:2 * tp + 2, h, :], ptv,
                        start=(j == 0), stop=(j == n - 1), perf_mode=DR)

                pend = []
                for j, tp in enumerate(pairs):
                    lane = LANES[lane_ctr[0] % len(LANES)]
                    lane_ctr[0] += 1
                    psc = pscpool.tile([128, 2, 512], F32, tag="psc")
                    for kt in (0, 1):
                        tt = 2 * tp + kt
                        nc.tensor.matmul(
                            psc[:, kt, :],
                            kT8[a0:a0 + 32, g0:g0 + 2, tt * 128:(tt + 1) * 128],
                            qT8[a0:a0 + 32, g0:g0 + 2, sc * 512:(sc + 1) * 512],
                            start=True, stop=True, perf_mode=DR,
                            tile_position=(a0, 0))
                    if lane == "A":
                        pt = wpool.tile([128, 2, 512], FP8, tag="ptA")
                        nc.scalar.activation(
                            pt[:], psc[:], mybir.ActivationFunctionType.Exp,
                            scale=0.125)
                        ptv = pt[:]
                    elif lane == "D":
                        pti = wpool.tile([128, 2, 512], I8, tag="ptD")
                        nc.vector.tensor_scalar(
                            pti[:], psc[:], A_SCH, B_SCH, MULT, ADD)
                        ptv = pti[:].bitcast(FP8)
                    else:  # lane P: DVE copies PSUM->SBUF bf16, Pool does sch
                        scf = wpool.tile([128, 2, 512], BF16, tag="scf")
                        nc.vector.tensor_copy(scf[:], psc[:])
                        pti = wpool.tile([128, 2, 512], I8, tag="ptP")
                        nc.gpsimd.tensor_scalar(
                            pti[:], scf[:], A_SCH, B_SCH, MULT, ADD)
                        ptv = pti[:].bitcast(FP8)
                    pend.append((j, tp, ptv))
                    if len(pend) == 3:
                        emit_pv(pend.pop(0))
                for item in pend:
                    emit_pv(item)

                slot = sc * 8 + h
                if is_group_a:
                    nc.vector.tensor_copy(yacc[:, slot, :], pav[0:65, :])
                else:
                    tmp = wpool.tile([65, 512], F32, tag="tmp")
                    nc.vector.tensor_tensor(
                        out=tmp[:], in0=yacc[:, slot, :], in1=pav[0:65, :],
                        op=ADD)
                    rec = wpool.tile([1, 512], F32, tag="rec")
                    nc.vector.reciprocal(rec[:], tmp[64:65, :])
                    rec_d = dpool.tile([1, 512], F32, tag="recd")
                    nc.sync.dma_start(rec_d[:], rec[:])
                    rrep = wpool.tile([64, 512], F32, tag="rrep")
                    nc.sync.dma_start(rrep[:], rec_d[:].to_broadcast((64, 512)))
                    nc.gpsimd.tensor_tensor(
                        out=yT2[0:64, h, sc * 512:(sc + 1) * 512],
                        in0=tmp[0:64, :], in1=rrep[:], op=MULT)

            def phase3_task(sc, sta):
                def f():
                    po = pscpool.tile([128, 2, 512], F32, tag="psc")
                    for i in range(2):
                        st = sc * 4 + sta * 2 + i
                        for h in range(H):
                            nc.tensor.matmul(
                                po[:, i, :], yT2[0:64, h, st * 128:(st + 1) * 128],
                                wo_s[0:64, h, :], start=(h == 0), stop=(h == 7))
                        o_s = wpool.tile([128, 512], F32, tag="osb")
                        nc.vector.tensor_tensor(out=o_s[:, :], in0=po[:, i, :],
                                                in1=bo_r[:], op=ADD)
                        nc.sync.dma_start(out_d[st * 128:(st + 1) * 128, :],
                                          o_s[:])
                return f

            # ---- emission ----
            emit_proj_chunk(0)
            emit_proj_chunk(1)
            emit_proj_chunk(2)
            # chunks 3..7 queued as fine-grained tasks, 2 per block boundary
            proj_queue = []
            for ch in range(3, 8):
                proj_queue.extend(make_proj_tasks(ch))
            blocks = [(sc, h) for sc in (0, 1) for h in range(H)]
            for bi, (sc, h) in enumerate(blocks):
                emit_block(h, sc, GROUP_A, True)
                for _ in range(2):
                    if proj_queue:
                        proj_queue.pop(0)()
            while proj_queue:
                proj_queue.pop(0)()
            # group B; spread each sc's output projection into the next sc's
            # blocks (final sc's at the end)
            pending_p3 = []
            for sc in (0, 1):
                for h in range(H):
                    emit_block(h, sc, GROUP_B, False)
                    if pending_p3:
                        pending_p3.pop(0)()
                pending_p3 = [phase3_task(sc, 0), phase3_task(sc, 1)]
            for t in pending_p3:
                t()
    nc.compile()
    return nc


_NC = None


def _pack_weights(Wq, bq, Wk, bk, Wv, bv, Wo, bo):
    s = lambda a: np.ascontiguousarray(np.asarray(a, np.float32))
    # e-permutation for DoubleRow plane layout: column c = eg*128+p of the
    # stationary maps to head h = p//32 + 4*(eg//2), e = 32*(eg%2) + p%32
    p = np.arange(128)
    eg = np.arange(4)
    hh = p[None, :] // 32 + 4 * (eg[:, None] // 2)     # [4,128]
    ee = 32 * (eg[:, None] % 2) + p[None, :] % 32      # [4,128]

    def pack_qk(W):
        t = np.asarray(W, np.float32)[hh, :, ee]       # [4,128,512(d)]
        t = t.transpose(2, 0, 1)                       # [d, eg, p]
        t = t.reshape(4, 128, 4, 128)                  # [ds, pd, eg, p]
        return s(t.transpose(1, 0, 2, 3).reshape(128, 4, 512))

    def pack_b(b):
        return s(np.asarray(b, np.float32)[hh, ee].T)  # [128,4]

    wq_p = pack_qk(Wq)
    wk_p = pack_qk(Wk)
    bq_p = pack_b(bq)
    bk_p = pack_b(bk)
    wv_p = s(np.transpose(Wv, (1, 0, 2)).reshape(D, 512).reshape(4, 128, 512)
             .transpose(1, 0, 2))
    wo_p = s(np.asarray(Wo, np.float32).reshape(8, 64, 512).transpose(1, 0, 2))
    bv_p = s(np.asarray(bv, np.float32).reshape(512))
    bo_p = s(np.asarray(bo, np.float32))
    return dict(wq=wq_p, wk=wk_p, wv=wv_p, wo=wo_p, bq=bq_p, bk=bk_p,
                bv=bv_p, bo=bo_p)


def kernel(x, Wq, bq, Wk, bk, Wv, bv, Wo, bo, **kw):
    global _NC
    x = np.asarray(x, np.float32)
    packed = _pack_weights(Wq, bq, Wk, bk, Wv, bv, Wo, bo)

    if _NC is None:
        _NC = build_program()

    in_maps = []
    for c in range(NCORES):
        b = c // 4
        q0 = (c % 4) * QCHUNK
        xb = np.roll(x[b], -q0, axis=0)  # queries at rows 0:1024
        m = {"xt": np.ascontiguousarray(xb.T)}
        m.update(packed)
        in_maps.append(m)
    res = run_bass_kernel_spmd(_NC, in_maps, core_ids=list(range(NCORES)))
    out = np.empty((B, S, D), np.float32)
    for c in range(NCORES):
        b = c // 4
        q0 = (c % 4) * QCHUNK
        out[b, q0:q0 + QCHUNK] = res.results[c]["out"]
    return out


# revision 19
# speedup vs baseline: 1.3569x; 1.0078x over previous
"""Multi-head attention Trainium2 kernel, 8-core SPMD (v2, fp8 DoubleRow).

Problem: x[2,4096,512], 8 heads of 64; per-head QKV proj, softmax(QK^T/8)V,
concat, output proj.

Sharding: sequence-parallel, no collectives. Core c handles batch b=c//4 and
query rows [1024*(c%4), ...+1024). Each core computes K/V for the full 4096-row
sequence of its batch; x is host-rolled so local query rows sit at 0:1024.

Speed structure:
- Q/K/V projections in f32r; K/Q converted to fp8-e4m3 with a host-side
  weight-column permutation so the PSUM->SBUF convert is partition-preserving
  and lands directly in the DoubleRow plane layout.
- Scores and PV use fp8-e4m3 DoubleRow matmuls (0.5 cyc/row): score stationary
  kT8[32,2,128] / moving qT8[32,2,512]; PV stationary vA[128,2,65] (ones column
  provides the softmax denominator) / moving pt[128,2,512].
- softmax exp is the wall; it is split across three engines by t-tile pair:
  lane A: ACT native Exp -> fp8 (scale=0.125 fused)
  lane D: DVE tensor_scalar computes e4m3 BITS of exp directly
          (Schraudolph: round(A*s+B) -> int8 -> bitcast fp8e4)
  lane P: DMA copies scores PSUM->SBUF, Pool does the same Schraudolph
- Attention is streamed in two t-groups (pairs 0-5, 6-15) so K/V projection of
  chunks 3..7 overlaps group-A attention; per-(h,sc) PV accumulates in one PSUM
  bank per group, group A drained to SBUF and merged at group-B end.
"""

import numpy as np

import concourse.bass as bass
from concourse import bacc
import concourse.mybir as mybir
import concourse.tile as tile
from concourse.bass_utils import run_bass_kernel_spmd

F32 = mybir.dt.float32
F32R = mybir.dt.float32r
BF16 = mybir.dt.bfloat16
FP8 = mybir.dt.float8e4
I8 = mybir.dt.int8
DR = mybir.MatmulPerfMode.DoubleRow
ADD = mybir.AluOpType.add
MULT = mybir.AluOpType.mult

B, S, D, H, E = 2, 4096, 512, 8, 64
NCORES = 8
QCHUNK = S // 4          # 1024 query rows per core
TCH = 512                # t-rows per projection chunk
NPAIR = S // 256         # 16 t-tile pairs
GROUP_A = list(range(7))        # chunks 0..3 (pair 6 needs chunk 3)
GROUP_B = list(range(7, NPAIR)) # chunks 3..7

A_SCH = 1.442695041      # 8*log2(e)*0.125
B_SCH = 55.632           # calibrated: max rel err 7.2% per weight

# exp lane pattern, cycled over all 256 (h,sc,pair) tasks: A=ACT native exp,
# D=DVE Schraudolph. (Pool cannot read PSUM and DMA cannot read PSUM, so a
# Pool lane would cost DVE the same feeder op as doing the exp on DVE.)
LANES = "DADADAADADAADADAADADAADADAADADAA"  # 19 A / 13 D per 32


def build_program():
    nc = bacc.Bacc()
    xt_d = nc.dram_tensor("xt", [D, S], F32R, kind="ExternalInput")
    wq_d = nc.dram_tensor("wq", [128, 4, 512], F32R, kind="ExternalInput")
    wk_d = nc.dram_tensor("wk", [128, 4, 512], F32R, kind="ExternalInput")
    wv_d = nc.dram_tensor("wv", [128, 4, 512], F32R, kind="ExternalInput")
    wo_d = nc.dram_tensor("wo", [64, 8, 512], F32R, kind="ExternalInput")
    bq_d = nc.dram_tensor("bq", [128, 4], F32, kind="ExternalInput")
    bk_d = nc.dram_tensor("bk", [128, 4], F32, kind="ExternalInput")
    bv_d = nc.dram_tensor("bv", [512], F32, kind="ExternalInput")
    bo_d = nc.dram_tensor("bo", [512], F32, kind="ExternalInput")
    out_d = nc.dram_tensor("out", [QCHUNK, D], F32, kind="ExternalOutput")

    with tile.TileContext(nc) as tc:
        with (
            tc.tile_pool(name="const", bufs=1) as cpool,
            tc.tile_pool(name="work", bufs=4) as wpool,
            tc.tile_pool(name="xtp", bufs=2) as xpool,
            tc.tile_pool(name="psc", bufs=3, space="PSUM") as pscpool,
            tc.tile_pool(name="pav", bufs=2, space="PSUM") as pavpool,
            tc.tile_pool(name="dr", bufs=2, space="DRAM") as dpool,
        ):
            wq_s = cpool.tile([128, 4, 512], F32R, tag="wq")
            wk_s = cpool.tile([128, 4, 512], F32R, tag="wk")
            wv_s = cpool.tile([128, 4, 512], F32R, tag="wv")
            wo_s = cpool.tile([64, 8, 512], F32R, tag="wo")
            bq_s = cpool.tile([128, 4], F32, tag="bq")
            bk_s = cpool.tile([128, 4], F32, tag="bk")
            bv_r = cpool.tile([128, 512], F32, tag="bvr")
            bo_r = cpool.tile([128, 512], F32, tag="bor")
            nc.sync.dma_start(wk_s[:], wk_d[:])
            nc.sync.dma_start(bk_s[:], bk_d[:])
            nc.sync.dma_start(wv_s[:], wv_d[:])
            nc.sync.dma_start(bv_r[:], bv_d[:].unsqueeze(0).to_broadcast((128, 512)))
            nc.sync.dma_start(wq_s[:], wq_d[:])
            nc.sync.dma_start(bq_s[:], bq_d[:])
            nc.sync.dma_start(bo_r[:], bo_d[:].unsqueeze(0).to_broadcast((128, 512)))
            nc.sync.dma_start(wo_s[:], wo_d[:])

            # fp8 operand tensors
            kT8 = cpool.tile([128, 4, S], FP8, tag="kT8")
            qT8 = cpool.tile([128, 4, QCHUNK], FP8, tag="qT8")
            vA = cpool.tile([128, S // 128, H, E + 2], FP8, tag="vA")
            yacc = cpool.tile([65, 16, 512], F32, tag="yacc")
            yT2 = cpool.tile([64, H, QCHUNK], F32R, tag="yT2")
            nc.vector.memset(vA[:, :, :, E], 1.0)
            nc.vector.memset(vA[:, :, :, E + 1], 0.0)

            def make_proj_tasks(ch):
                """Per-chunk projection split into psc-ring-sized tasks so it
                can interleave with attention blocks at fine grain."""
                state = {}

                def t_x():
                    xT = xpool.tile([128, 4, TCH], F32R, tag="xT")
                    for ds in range(4):
                        nc.sync.dma_start(
                            xT[:, ds, :],
                            xt_d[ds * 128:(ds + 1) * 128,
                                 ch * TCH:(ch + 1) * TCH])
                    state["xT"] = xT

                def t_k(ega):
                    def f():
                        xT = state["xT"]
                        pk = pscpool.tile([128, 2, 512], F32, tag="psc")
                        for i in range(2):
                            eg = ega * 2 + i
                            for ds in range(4):
                                nc.tensor.matmul(
                                    pk[:, i, :],
                                    wk_s[:, ds, eg * 128:(eg + 1) * 128],
                                    xT[:, ds, :], start=(ds == 0),
                                    stop=(ds == 3))
                            nc.scalar.activation(
                                kT8[:, eg, ch * TCH:(ch + 1) * TCH],
                                pk[:, i, :],
                                mybir.ActivationFunctionType.Identity,
                                bias=bk_s[:, eg:eg + 1], scale=1.0)
                    return f

                def t_v(tsa):
                    def f():
                        xT = state["xT"]
                        pv = pscpool.tile([128, 2, 512], F32, tag="psc")
                        for i in range(2):
                            ts = tsa * 2 + i
                            for ds in range(4):
                                nc.tensor.matmul(
                                    pv[:, i, :],
                                    xT[:, ds, ts * 128:(ts + 1) * 128],
                                    wv_s[:, ds, :], start=(ds == 0),
                                    stop=(ds == 3))
                            nc.vector.tensor_tensor(
                                out=vA[:, ch * 4 + ts, :, 0:E],
                                in0=pv[:, i, :].rearrange("p (h e) -> p h e",
                                                          h=H),
                                in1=bv_r[:].rearrange("p (h e) -> p h e", h=H),
                                op=ADD)
                    return f

                def t_q(ega):
                    def f():
                        xT = state["xT"]
                        pq = pscpool.tile([128, 2, 512], F32, tag="psc")
                        for i in range(2):
                            eg = ega * 2 + i
                            for ds in range(4):
                                nc.tensor.matmul(
                                    pq[:, i, :],
                                    wq_s[:, ds, eg * 128:(eg + 1) * 128],
                                    xT[:, ds, :], start=(ds == 0),
                                    stop=(ds == 3))
                            nc.vector.tensor_scalar(
                                qT8[:, eg, ch * TCH:(ch + 1) * TCH],
                                pq[:, i, :], bq_s[:, eg:eg + 1], None, ADD)
                    return f

                def first():
                    t_x()
                    t_k(0)()

                if ch < QCHUNK // TCH:
                    # queries first so attention can start early
                    tasks = [first, t_q(0), t_v(0), t_k(1), t_v(1), t_q(1)]
                else:
                    tasks = [first, t_v(0), t_k(1), t_v(1)]
                return tasks

            def emit_proj_chunk(ch):
                for t in make_proj_tasks(ch):
                    t()

            lane_ctr = [0]

            def emit_block(h, sc, pairs, is_group_a):
                a0 = 32 * (h % 4)
                g0 = 2 * (h // 4)
                n = len(pairs)
                pav = pavpool.tile([128, 512], F32, tag="pav")

                def emit_pv(item):
                    j, tp, ptv = item
                    nc.tensor.matmul(
                        pav[0:66, :], vA[:, 2 * tp:2 * tp + 2, h, :], ptv,
                        start=(j == 0), stop=(j == n - 1), perf_mode=DR)

                pend = []
                for j, tp in enumerate(pairs):
                    lane = LANES[lane_ctr[0] % len(LANES)]
                    lane_ctr[0] += 1
                    psc = pscpool.tile([128, 2, 512], F32, tag="psc")
                    for kt in (0, 1):
                        tt = 2 * tp + kt
                        nc.tensor.matmul(
                            psc[:, kt, :],
                            kT8[a0:a0 + 32, g0:g0 + 2, tt * 128:(tt + 1) * 128],
                            qT8[a0:a0 + 32, g0:g0 + 2, sc * 512:(sc + 1) * 512],
                            start=True, stop=True, perf_mode=DR,
                            tile_position=(a0, 0))
                    if lane == "A":
                        pt = wpool.tile([128, 2, 512], FP8, tag="ptA")
                        nc.scalar.activation(
                            pt[:], psc[:], mybir.ActivationFunctionType.Exp,
                            scale=0.125)
                        ptv = pt[:]
                    elif lane == "D":
                        pti = wpool.tile([128, 2, 512], I8, tag="ptD")
                        nc.vector.tensor_scalar(
                            pti[:], psc[:], A_SCH, B_SCH, MULT, ADD)
                        ptv = pti[:].bitcast(FP8)
                    else:  # lane P: DVE copies PSUM->SBUF bf16, Pool does sch
                        scf = wpool.tile([128, 2, 512], BF16, tag="scf")
                        nc.vector.tensor_copy(scf[:], psc[:])
                        pti = wpool.tile([128, 2, 512], I8, tag="ptP")
                        nc.gpsimd.tensor_scalar(
                            pti[:], scf[:], A_SCH, B_SCH, MULT, ADD)
                        ptv = pti[:].bitcast(FP8)
                    pend.append((j, tp, ptv))
                    if len(pend) == 3:
                        emit_pv(pend.pop(0))
                for item in pend:
                    emit_pv(item)

                slot = sc * 8 + h
                if is_group_a:
                    nc.vector.tensor_copy(yacc[:, slot, :], pav[0:65, :])
                else:
                    tmp = wpool.tile([65, 512], F32, tag="tmp")
                    nc.vector.tensor_tensor(
                        out=tmp[:], in0=yacc[:, slot, :], in1=pav[0:65, :],
                        op=ADD)
                    rec = wpool.tile([1, 512], F32, tag="rec")
                    nc.vector.reciprocal(rec[:], tmp[64:65, :])
                    rec_d = dpool.tile([1, 512], F32, tag="recd")
                    nc.sync.dma_start(rec_d[:], rec[:])
                    rrep = wpool.tile([64, 512], F32, tag="rrep")
                    nc.sync.dma_start(rrep[:], rec_d[:].to_broadcast((64, 512)))
                    nc.gpsimd.tensor_tensor(
                        out=yT2[0:64, h, sc * 512:(sc + 1) * 512],
                        in0=tmp[0:64, :], in1=rrep[:], op=MULT)

            def phase3_task(sc, sta):
                def f():
                    po = pscpool.tile([128, 2, 512], F32, tag="psc")
                    for i in range(2):
                        st = sc * 4 + sta * 2 + i
                        for h in range(H):
                            nc.tensor.matmul(
                                po[:, i, :], yT2[0:64, h, st * 128:(st + 1) * 128],
                                wo_s[0:64, h, :], start=(h == 0), stop=(h == 7))
                        o_s = wpool.tile([128, 512], F32, tag="osb")
                        nc.vector.tensor_tensor(out=o_s[:, :], in0=po[:, i, :],
                                                in1=bo_r[:], op=ADD)
                        nc.sync.dma_start(out_d[st * 128:(st + 1) * 128, :],
                                          o_s[:])
                return f

            # ---- emission ----
            emit_proj_chunk(0)
            emit_proj_chunk(1)
            emit_proj_chunk(2)
            emit_proj_chunk(3)  # group A reaches pair 6 = chunk 3
            # chunks 4..7 queued as fine-grained tasks, 2 per block boundary
            proj_queue = []
            for ch in range(4, 8):
                proj_queue.extend(make_proj_tasks(ch))
            blocks = [(sc, h) for sc in (0, 1) for h in range(H)]
            for bi, (sc, h) in enumerate(blocks):
                emit_block(h, sc, GROUP_A, True)
                for _ in range(2):
                    if proj_queue:
                        proj_queue.pop(0)()
            while proj_queue:
                proj_queue.pop(0)()
            # group B; spread each sc's output projection into the next sc's
            # blocks (final sc's at the end)
            pending_p3 = []
            for sc in (0, 1):
                for h in range(H):
                    emit_block(h, sc, GROUP_B, False)
                    if pending_p3:
                        pending_p3.pop(0)()
                pending_p3 = [phase3_task(sc, 0), phase3_task(sc, 1)]
            for t in pending_p3:
                t()
    nc.compile()
    return nc


_NC = None


def _pack_weights(Wq, bq, Wk, bk, Wv, bv, Wo, bo):
    s = lambda a: np.ascontiguousarray(np.asarray(a, np.float32))
    # e-permutation for DoubleRow plane layout: column c = eg*128+p of the
    # stationary maps to head h = p//32 + 4*(eg//2), e = 32*(eg%2) + p%32
    p = np.arange(128)
    eg = np.arange(4)
    hh = p[None, :] // 32 + 4 * (eg[:, None] // 2)     # [4,128]
    ee = 32 * (eg[:, None] % 2) + p[None, :] % 32      # [4,128]

    def pack_qk(W):
        t = np.asarray(W, np.float32)[hh, :, ee]       # [4,128,512(d)]
        t = t.transpose(2, 0, 1)                       # [d, eg, p]
        t = t.reshape(4, 128, 4, 128)                  # [ds, pd, eg, p]
        return s(t.transpose(1, 0, 2, 3).reshape(128, 4, 512))

    def pack_b(b):
        return s(np.asarray(b, np.float32)[hh, ee].T)  # [128,4]

    wq_p = pack_qk(Wq)
    wk_p = pack_qk(Wk)
    bq_p = pack_b(bq)
    bk_p = pack_b(bk)
    wv_p = s(np.transpose(Wv, (1, 0, 2)).reshape(D, 512).reshape(4, 128, 512)
             .transpose(1, 0, 2))
    wo_p = s(np.asarray(Wo, np.float32).reshape(8, 64, 512).transpose(1, 0, 2))
    bv_p = s(np.asarray(bv, np.float32).reshape(512))
    bo_p = s(np.asarray(bo, np.float32))
    return dict(wq=wq_p, wk=wk_p, wv=wv_p, wo=wo_p, bq=bq_p, bk=bk_p,
                bv=bv_p, bo=bo_p)


def kernel(x, Wq, bq, Wk, bk, Wv, bv, Wo, bo, **kw):
    global _NC
    x = np.asarray(x, np.float32)
    packed = _pack_weights(Wq, bq, Wk, bk, Wv, bv, Wo, bo)

    if _NC is None:
        _NC = build_program()

    in_maps = []
    for c in range(NCORES):
        b = c // 4
        q0 = (c % 4) * QCHUNK
        xb = np.roll(x[b], -q0, axis=0)  # queries at rows 0:1024
        m = {"xt": np.ascontiguousarray(xb.T)}
        m.update(packed)
        in_maps.append(m)
    res = run_bass_kernel_spmd(_NC, in_maps, core_ids=list(range(NCORES)))
    out = np.empty((B, S, D), np.float32)
    for c in range(NCORES):
        b = c // 4
        q0 = (c % 4) * QCHUNK
        out[b, q0:q0 + QCHUNK] = res.results[c]["out"]
    return out
